# revision 1
# baseline (speedup 1.0000x reference)
"""Trainium2 Bass kernel for nn_DetectorKe_652835029279 (Gaussian-mixture
log-likelihood detector: weighted logsumexp over 256 Mahalanobis distances).

Math: ll_i = log sum_j coef_j * exp(-0.5 * (x_i-c_j)^T A_j (x_i-c_j)) - thr
    = logsumexp_j( -0.5 * x^T A_j x + x . (A_j c_j) + bias_j )
with bias_j = log(coef_j) - 0.5 c_j^T A_j c_j - thr folded in, and the
quadratic term expanded over the 17 cyclic-rotation pair blocks
(d, (d+k) % 32), k = 0..16 (544 pair slots; upper-triangle coverage with
doubled off-diagonal coefficients), so the whole row reduces to ONE matmul
  d'[i, j] = sum_s G[i, s] * U[s, j]
with G = [x_a * x_b (544 slots), x (32), 1, zero-pad] built on-chip and U
precomputed on host (tiny, M-sized).

All matmuls are float32r (fp22 read-truncation, ~1 cycle/row) and K-padded
to 128 partitions (K<128 runs at half rate on trn2) - pad rows are exact
zeros on both operands so they contribute nothing.

Device layout per core (data-parallel over N, 16384 rows/core), per
512-row tile: DMA X -> 4 PE transposes to X^T [32,512] -> 6 padded
selection matmuls build rotated copies -> 5 DVE multiplies build the pair
products -> 24 accumulating K=128 matmuls (chunk-outer order, one PSUM
wait per chunk) into PSUM [128,1024] -> ACT exp with fused free-dim
accumulate -> Ln + PE transpose + contiguous DMA out at the end.
"""
import sys

if "/opt/trn_rl_repo" not in sys.path:
    sys.path.insert(0, "/opt/trn_rl_repo")

import numpy as np

N, D, M = 131072, 32, 256
NCORES = 8
NC_ROWS = N // NCORES          # 16384
TILE_ROWS = 512
NTILES = NC_ROWS // TILE_ROWS  # 32
NGROUPS = NC_ROWS // 128       # 128
NCHUNK = 6

_PROGRAM = None


def _build_program():
    import concourse.bacc as bacc
    import concourse.mybir as mybir
    import concourse.tile as tile

    f32 = mybir.dt.float32
    f32r = mybir.dt.float32r
    AF = mybir.ActivationFunctionType

    nc = bacc.Bacc(None, target_bir_lowering=False)
    X_d = nc.dram_tensor("X", [NC_ROWS, D], f32r, kind="ExternalInput")
    U_d = nc.dram_tensor("U", [128, NCHUNK, M], f32r, kind="ExternalInput")
    SEL_d = nc.dram_tensor("SEL", [128, 768], f32r, kind="ExternalInput")
    PAD_d = nc.dram_tensor("PAD", [96, TILE_ROWS], f32r, kind="ExternalInput")
    EYE_d = nc.dram_tensor("EYE", [128, 128], f32, kind="ExternalInput")
    EYER_d = nc.dram_tensor("EYER", [128, 128], f32r, kind="ExternalInput")
    OUT_d = nc.dram_tensor("out", [NC_ROWS], f32, kind="ExternalOutput")

    with tile.TileContext(nc) as tc:
        with (
            tc.tile_pool(name="const", bufs=1) as constp,
            tc.tile_pool(name="xin", bufs=3) as xinp,
            tc.tile_pool(name="xtp", bufs=2) as xtpool,
            tc.tile_pool(name="xt4p", bufs=2) as xt4pool,
            tc.tile_pool(name="xxp", bufs=2) as xxpool,
            tc.tile_pool(name="expp", bufs=4) as exppool,
            tc.tile_pool(name="sumsp", bufs=1) as sumspool,
            tc.tile_pool(name="finp", bufs=1) as finpool,
            tc.tile_pool(name="ps_xt", bufs=2, space="PSUM") as ps_xt,
            tc.tile_pool(name="ps_xt4", bufs=1, space="PSUM") as ps_xt4,
            tc.tile_pool(name="ps_rot", bufs=2, space="PSUM") as ps_rot,
            tc.tile_pool(name="ps_main", bufs=3, space="PSUM") as ps_main,
        ):
            U_sb = constp.tile([128, NCHUNK, M], f32r)
            nc.sync.dma_start(U_sb[:], U_d[:])
            SEL_sb = constp.tile([128, 768], f32r)
            nc.sync.dma_start(SEL_sb[:], SEL_d[:])
            EYE_sb = constp.tile([128, 128], f32)
            nc.sync.dma_start(EYE_sb[:], EYE_d[:])
            EYER_sb = constp.tile([128, 128], f32r)
            nc.sync.dma_start(EYER_sb[:], EYER_d[:])

            sums_sb = sumspool.tile([128, NGROUPS], f32)

            # persistent double-buffered X^T tiles: rows 32:128 hold the
            # constant [ones-row; zeros] pad, DMA'd once - per-tile writes
            # only touch rows 0:32, so the pad stays valid across reuse.
            xt_tiles = []
            for i in range(2):
                xt_p = xtpool.tile(
                    [128, TILE_ROWS], f32r, tag=f"xtP{i}", bufs=1, name=f"xt_p{i}"
                )
                nc.sync.dma_start(xt_p[32:128, :], PAD_d[:])
                xt_tiles.append(xt_p)

            for t in range(NTILES):
                x_t = xinp.tile([128, 4 * D], f32r, tag="x")
                nc.sync.dma_start(
                    x_t[:].rearrange("p (g d) -> p g d", g=4),
                    X_d[t * TILE_ROWS : (t + 1) * TILE_ROWS, :].rearrange(
                        "(g p) d -> p g d", p=128
                    ),
                )

                # X^T [32, 512] via 4 PE transposes
                xtps = ps_xt.tile([32, TILE_ROWS], f32r, tag="xtps")
                for g in range(4):
                    nc.tensor.transpose(
                        xtps[:, g * 128 : (g + 1) * 128],
                        x_t[:, g * D : (g + 1) * D],
                        EYER_sb[:],
                    )
                # xt_sb = [X^T (32) ; ones (1) ; zeros (95)] - serves both as
                # the sel-matmul moving operand (rows 32:128 exactly zero) and
                # as main-matmul chunk 5 (x-linear part + bias row).
                xt_sb = xt_tiles[t % 2]
                nc.scalar.copy(xt_sb[:32, :], xtps[:])

                # XT4 = 4-fold stack of X^T (partition p holds x_{p%32})
                xt4ps = ps_xt4.tile([128, TILE_ROWS], f32, tag="xt4ps")
                nc.tensor.matmul(
                    xt4ps[:], SEL_sb[:, 0:128], xt_sb[:], start=True, stop=True
                )
                xt4_sb = xt4pool.tile([128, TILE_ROWS], f32r, tag="xt4")
                nc.scalar.copy(xt4_sb[:], xt4ps[:])

                # pair-product chunks 0..3:
                #   chunk_c[p] = x_{p%32} * x_{(p%32 + 4c + p//32)%32}
                # chunk 4: k=16 block in rows 0:32, rows 32:128 exact zeros
                # (sel rows are zero there, and xt4 * 0 = 0).
                chunk_tiles = []
                for c in range(5):
                    rotps = ps_rot.tile([128, TILE_ROWS], f32, tag="rot")
                    nc.tensor.matmul(
                        rotps[:],
                        SEL_sb[:, 128 * (c + 1) : 128 * (c + 2)],
                        xt_sb[:],
                        start=True,
                        stop=True,
                    )
                    xx_c = xxpool.tile([128, TILE_ROWS], f32r, tag=f"xx{c}")
                    nc.vector.tensor_mul(xx_c[:], xt4_sb[:], rotps[:])
                    chunk_tiles.append(xx_c)
                chunk_tiles.append(xt_sb)  # chunk 5: [X^T; ones; zeros]

                # main accumulating matmuls (one open PSUM group per bank);
                # two 1-bank psum tiles (2 row-groups each) for deeper overlap
                for half in range(2):
                    psmain = ps_main.tile([128, 2 * M], f32, tag="main")
                    for s2 in range(2):
                        sub = half * 2 + s2
                        for c in range(NCHUNK):
                            nc.tensor.matmul(
                                psmain[:, s2 * M : (s2 + 1) * M],
                                chunk_tiles[c][:, sub * 128 : (sub + 1) * 128],
                                U_sb[:, c, :],
                                start=(c == 0),
                                stop=(c == NCHUNK - 1),
                            )
                    for s2 in range(2):
                        sub = half * 2 + s2
                        expsc = exppool.tile([128, M], f32, tag="exp")
                        col = t * 4 + sub
                        nc.scalar.activation(
                            expsc[:],
                            psmain[:, s2 * M : (s2 + 1) * M],
                            AF.Exp,
                            accum_out=sums_sb[:, col : col + 1],
                        )

            # epilogue: ll^T = Ln(sums); transpose; contiguous DMA out
            llT = finpool.tile([128, NGROUPS], f32)
            nc.scalar.activation(llT[:], sums_sb[:], AF.Ln)
            llps = ps_xt.tile([128, 128], f32, tag="xtps")
            nc.tensor.transpose(llps[:], llT[:], EYE_sb[:])
            ll_sb = finpool.tile([128, 128], f32)
            nc.scalar.copy(ll_sb[:], llps[:])
            nc.sync.dma_start(OUT_d.rearrange("(c p) -> c p", c=128), ll_sb[:])

    nc.compile()
    return nc


def _host_prep(center, cov_inv_sqrt, weight, threshold):
    L = np.asarray(cov_inv_sqrt, dtype=np.float64)
    w = np.abs(np.asarray(weight, dtype=np.float64))
    pr = w / w.sum()
    A = np.einsum("mij,mkj->mik", L, L)
    sign, logdet = np.linalg.slogdet(A)
    logcoef = np.log(pr) + 0.5 * logdet
    c64 = np.asarray(center, dtype=np.float64)
    Ac = np.einsum("mkl,ml->mk", A, c64)
    term3 = np.einsum("mk,mk->m", c64, Ac)
    bias = logcoef - 0.5 * term3 - float(np.asarray(threshold).reshape(-1)[0])

    U = np.zeros((128, NCHUNK, M), np.float32)
    p = np.arange(128)
    for c in range(4):
        k = 4 * c + p // 32
        d1 = p % 32
        d2 = (d1 + k) % 32
        mult = np.where((k == 0) | (k == 16), 1.0, 2.0)
        U[:, c, :] = (-0.5 * mult[:, None] * A[:, d1, d2].T).astype(np.float32)
    p32 = np.arange(32)
    U[:32, 4, :] = (-0.5 * A[:, p32, (p32 + 16) % 32].T).astype(np.float32)
    U[:32, 5, :] = Ac.T.astype(np.float32)
    U[32, 5, :] = bias.astype(np.float32)

    SEL = np.zeros((128, 768), np.float32)
    dd = np.arange(128)
    SEL[:, 0:128] = (dd[:, None] == (p[None, :] % 32)).astype(np.float32)
    for c in range(4):
        k = 4 * c + p // 32
        b = (p % 32 + k) % 32
        SEL[:, 128 * (c + 1) : 128 * (c + 2)] = (dd[:, None] == b[None, :]).astype(
            np.float32
        )
    b16 = np.where(p < 32, (p + 16) % 32, -1)
    SEL[:, 640:768] = (dd[:, None] == b16[None, :]).astype(np.float32)

    PAD = np.zeros((96, TILE_ROWS), np.float32)
    PAD[0, :] = 1.0
    EYE = np.eye(128, dtype=np.float32)
    return U, SEL, PAD, EYE


def kernel(X, center, cov_inv_sqrt, weight, threshold):
    global _PROGRAM
    from concourse.bass_utils import run_bass_kernel_spmd

    X = np.ascontiguousarray(np.asarray(X, dtype=np.float32))
    U, SEL, PAD, EYE = _host_prep(center, cov_inv_sqrt, weight, threshold)

    if _PROGRAM is None:
        _PROGRAM = _build_program()
    nc = _PROGRAM

    in_maps = []
    for k in range(NCORES):
        in_maps.append(
            {
                "X": X[k * NC_ROWS : (k + 1) * NC_ROWS],
                "U": U,
                "SEL": SEL,
                "PAD": PAD,
                "EYE": EYE,
                "EYER": EYE,
            }
        )
    res = run_bass_kernel_spmd(nc, in_maps, list(range(NCORES)))
    out = np.concatenate([res.results[k]["out"] for k in range(NCORES)])
    return out.astype(np.float32)



# revision 4
# speedup vs baseline: 1.3222x; 1.3222x over previous
"""Trainium2 Bass kernel for nn_DetectorKe_652835029279 (Gaussian-mixture
log-likelihood detector: weighted logsumexp over 256 Mahalanobis distances).

Math: ll_i = logsumexp_j( -0.5 x^T A_j x + x . (A_j c_j) + bias_j )
with bias_j = log(coef_j) - 0.5 c_j^T A_j c_j - thr folded in. The quadratic
expands over cyclic-rotation pair blocks (d, (d+k) % 32):
  chunks 0..3 (bf16): k = 1..16 pair products (512 slots, off-diag, doubled
    coefficients except k=16), built as xt4 * rot via DVE;
  chunk 4 (bf16): [diag squares x_d^2 (32); x_d linear (32); 1 bias row;
    zero pad] - squares via DVE (xt4*xt4, 2x bf16 mode), linear via GpSimd
    copy, ones/zero rows static.
Main matmul: per 128-row subtile, 5 accumulating matmuls, stationary = bf16
G-chunk [128,128] (FWL fast weight load), moving = bf16 U [128,256].
Rotations: 4 concurrent K=32 row-tiled sel matmuls (tile_position=(32g,0))
reading xt4 partition strips; xt4 itself via one K=128 sel matmul.
Software-pipelined: tile t+1's construction is emitted before tile t's main
matmuls so DVE/scalar/gpsimd overlap the PE main phase.

Engine budget/tile: PE ~3.0us, DVE (xt copy + big mult) ~2.9us, scalar
(xt4 copy + 4x exp-accum) ~3.0us, gpsimd (square + linear copy) ~1.6us.
"""
import sys

if "/opt/trn_rl_repo" not in sys.path:
    sys.path.insert(0, "/opt/trn_rl_repo")

import numpy as np

N, D, M = 131072, 32, 256
NCORES = 8
NC_ROWS = N // NCORES          # 16384
TILE_ROWS = 512
NTILES = NC_ROWS // TILE_ROWS  # 32
NGROUPS = NC_ROWS // 128       # 128
NCHUNK = 5
KL = [[1, 2, 3, 4], [5, 6, 7, 8], [9, 10, 11, 12], [13, 14, 15, 16]]

_PROGRAM = None


def _build_program():
    import concourse.bacc as bacc
    import concourse.mybir as mybir
    import concourse.tile as tile

    f32 = mybir.dt.float32
    f32r = mybir.dt.float32r
    bf16 = mybir.dt.bfloat16
    AF = mybir.ActivationFunctionType

    nc = bacc.Bacc(None, target_bir_lowering=False)
    X_d = nc.dram_tensor("X", [NC_ROWS, D], f32r, kind="ExternalInput")
    U_d = nc.dram_tensor("U", [128, NCHUNK, M], bf16, kind="ExternalInput")
    SEL0_d = nc.dram_tensor("SEL0", [128, 128], bf16, kind="ExternalInput")
    SELR_d = nc.dram_tensor("SELR", [128, 128], bf16, kind="ExternalInput")
    EYER_d = nc.dram_tensor("EYER", [128, 128], f32r, kind="ExternalInput")
    EYE_d = nc.dram_tensor("EYE", [128, 128], f32, kind="ExternalInput")
    OUT_d = nc.dram_tensor("out", [NC_ROWS], f32, kind="ExternalOutput")

    with tile.TileContext(nc) as tc:
        with (
            tc.tile_pool(name="const", bufs=1) as constp,
            tc.tile_pool(name="xin", bufs=3) as xinp,
            tc.tile_pool(name="xtp", bufs=2) as xtpool,
            tc.tile_pool(name="xt4p", bufs=2) as xt4pool,
            tc.tile_pool(name="ch4p", bufs=2) as ch4pool,
            tc.tile_pool(name="xxp", bufs=2) as xxpool,
            tc.tile_pool(name="expp", bufs=4) as exppool,
            tc.tile_pool(name="sumsp", bufs=1) as sumspool,
            tc.tile_pool(name="finp", bufs=1) as finpool,
            tc.tile_pool(name="ps_xt", bufs=1, space="PSUM") as ps_xt,
            tc.tile_pool(name="ps_xt4", bufs=1, space="PSUM") as ps_xt4,
            tc.tile_pool(name="ps_rot", bufs=1, space="PSUM") as ps_rot,
            tc.tile_pool(name="ps_main", bufs=2, space="PSUM") as ps_main,
        ):
            U_sb = constp.tile([128, NCHUNK, M], bf16)
            nc.sync.dma_start(U_sb[:], U_d[:])
            SEL0_sb = constp.tile([128, 128], bf16)
            nc.sync.dma_start(SEL0_sb[:], SEL0_d[:])
            SELR_sb = constp.tile([128, 128], bf16)
            nc.sync.dma_start(SELR_sb[:], SELR_d[:])
            EYER_sb = constp.tile([128, 128], f32r)
            nc.sync.dma_start(EYER_sb[:], EYER_d[:])
            EYE_sb = constp.tile([128, 128], f32)
            nc.sync.dma_start(EYE_sb[:], EYE_d[:])

            sums_sb = sumspool.tile([128, NGROUPS], f32)

            # persistent tiles: static pad regions written once, per-tile
            # writes only touch the dynamic rows so statics stay valid.
            xt_tiles = []
            for i in range(2):
                xt_p = xtpool.tile(
                    [128, TILE_ROWS], bf16, tag=f"xtP{i}", bufs=1, name=f"xt_p{i}"
                )
                for lo in (32, 64, 96):  # sel pad rows (quadrant-aligned memsets)
                    nc.vector.memset(xt_p[lo : lo + 32, :], 0.0)
                xt_tiles.append(xt_p)
            ch4_tiles = []
            for i in range(2):
                ch4_p = ch4pool.tile(
                    [128, TILE_ROWS], bf16, tag=f"ch4P{i}", bufs=1, name=f"ch4_p{i}"
                )
                nc.gpsimd.memset(ch4_p[64:96, :], 0.0)   # zero pad (U rows are 0)
                nc.gpsimd.memset(ch4_p[96:128, :], 0.0)
                nc.gpsimd.memset(ch4_p[64:65, :], 1.0)   # bias row
                ch4_tiles.append(ch4_p)

            def construct(t):
                """Emit chunk-construction for tile t (PE sels + DVE/gpsimd)."""
                x_t = xinp.tile([128, 4 * D], f32r, tag="x")
                nc.sync.dma_start(
                    x_t[:].rearrange("p (g d) -> p g d", g=4),
                    X_d[t * TILE_ROWS : (t + 1) * TILE_ROWS, :].rearrange(
                        "(g p) d -> p g d", p=128
                    ),
                )
                # X^T [32, 512] via 4 PE transposes
                xtps = ps_xt.tile([32, TILE_ROWS], f32r, tag="xtps")
                for g in range(4):
                    nc.tensor.transpose(
                        xtps[:, g * 128 : (g + 1) * 128],
                        x_t[:, g * D : (g + 1) * D],
                        EYER_sb[:],
                    )
                xt_sb = xt_tiles[t % 2]
                nc.vector.tensor_copy(xt_sb[0:32, :], xtps[:])  # f32->bf16

                # xt4 = 4-fold stack of X^T (partition p holds x_{p%32})
                xt4ps = ps_xt4.tile([128, TILE_ROWS], f32, tag="xt4ps")
                nc.tensor.matmul(
                    xt4ps[:], SEL0_sb[:], xt_sb[:], start=True, stop=True
                )
                xt4_sb = xt4pool.tile([128, TILE_ROWS], bf16, tag="xt4")
                nc.scalar.copy(xt4_sb[:], xt4ps[:])

                # 4 rotation sels, K=32 row-tiled (concurrent in PE array):
                # rot_g[p] = x_{(p%32 + KL[g][p//32]) % 32}
                rotq = ps_rot.tile([128, 4, TILE_ROWS], f32, tag="rotq")
                for g in range(4):
                    nc.tensor.matmul(
                        rotq[:, g, :],
                        SELR_sb[32 * g : 32 * (g + 1), :],
                        xt4_sb[32 * g : 32 * (g + 1), :],
                        start=True,
                        stop=True,
                        tile_position=(32 * g, 0),
                    )

                # chunk 4 dynamic rows: squares (DVE 2x bf16) + linear (gpsimd)
                ch4 = ch4_tiles[t % 2]
                nc.vector.tensor_mul(ch4[0:32, :], xt_sb[0:32, :], xt_sb[0:32, :])
                nc.gpsimd.tensor_copy(ch4[32:64, :], xt_sb[0:32, :])

                # pair products for chunks 0..3, one DVE instruction
                xx = xxpool.tile([128, 4, TILE_ROWS], bf16, tag="xx")
                nc.vector.tensor_mul(
                    xx[:],
                    rotq[:],
                    xt4_sb[:, None, :].broadcast_to([128, 4, TILE_ROWS]),
                )
                return xx, ch4

            def main_phase(t, xx, ch4):
                for half in range(2):
                    psmain = ps_main.tile([128, 2 * M], f32, tag="main")
                    for s2 in range(2):
                        sub = half * 2 + s2
                        for c in range(NCHUNK):
                            lhsT = (
                                xx[:, c, sub * 128 : (sub + 1) * 128]
                                if c < 4
                                else ch4[:, sub * 128 : (sub + 1) * 128]
                            )
                            nc.tensor.matmul(
                                psmain[:, s2 * M : (s2 + 1) * M],
                                lhsT,
                                U_sb[:, c, :],
                                start=(c == 0),
                                stop=(c == NCHUNK - 1),
                            )
                    for s2 in range(2):
                        sub = half * 2 + s2
                        expsc = exppool.tile([128, M], f32, tag="exp")
                        col = t * 4 + sub
                        nc.scalar.activation(
                            expsc[:],
                            psmain[:, s2 * M : (s2 + 1) * M],
                            AF.Exp,
                            accum_out=sums_sb[:, col : col + 1],
                        )

            # software pipeline: construct t+1 ahead of main t
            prev = construct(0)
            for t in range(NTILES):
                nxt = construct(t + 1) if t + 1 < NTILES else None
                main_phase(t, *prev)
                prev = nxt

            # epilogue: ll^T = Ln(sums); transpose; contiguous DMA out
            llT = finpool.tile([128, NGROUPS], f32)
            nc.scalar.activation(llT[:], sums_sb[:], AF.Ln)
            llps = ps_xt4.tile([128, 128], f32, tag="xt4ps")
            nc.tensor.transpose(llps[:], llT[:], EYE_sb[:])
            ll_sb = finpool.tile([128, 128], f32)
            nc.scalar.copy(ll_sb[:], llps[:])
            nc.sync.dma_start(OUT_d.rearrange("(c p) -> c p", c=128), ll_sb[:])

    nc.compile()
    return nc


def _host_prep(center, cov_inv_sqrt, weight, threshold):
    import ml_dtypes

    bf = ml_dtypes.bfloat16
    L = np.asarray(cov_inv_sqrt, dtype=np.float64)
    w = np.abs(np.asarray(weight, dtype=np.float64))
    pr = w / w.sum()
    A = np.einsum("mij,mkj->mik", L, L)
    sign, logdet = np.linalg.slogdet(A)
    logcoef = np.log(pr) + 0.5 * logdet
    c64 = np.asarray(center, dtype=np.float64)
    Ac = np.einsum("mkl,ml->mk", A, c64)
    term3 = np.einsum("mk,mk->m", c64, Ac)
    bias = logcoef - 0.5 * term3 - float(np.asarray(threshold).reshape(-1)[0])

    p = np.arange(128)
    U = np.zeros((128, NCHUNK, M), np.float32)
    for c in range(4):
        k = np.array(KL[c])[p // 32]
        a = p % 32
        b = (a + k) % 32
        mult = np.where(k == 16, 1.0, 2.0)
        U[:, c, :] = -0.5 * mult[:, None] * A[:, a, b].T
    d32 = np.arange(32)
    U[0:32, 4, :] = -0.5 * A[:, d32, d32].T
    U[32:64, 4, :] = Ac.T
    U[64, 4, :] = bias
    U = U.astype(bf)

    dd = np.arange(128)
    SEL0 = (dd[:, None] == (p[None, :] % 32)).astype(bf)
    SELR = np.zeros((128, 128), np.float32)
    for g in range(4):
        k = np.array(KL[g])[p // 32]
        b = (p % 32 + k) % 32
        SELR[32 * g : 32 * (g + 1), :] = (
            np.arange(32)[:, None] == b[None, :]
        ).astype(np.float32)
    SELR = SELR.astype(bf)
    EYE = np.eye(128, dtype=np.float32)
    return U, SEL0, SELR, EYE


def kernel(X, center, cov_inv_sqrt, weight, threshold):
    global _PROGRAM
    from concourse.bass_utils import run_bass_kernel_spmd

    X = np.ascontiguousarray(np.asarray(X, dtype=np.float32))
    U, SEL0, SELR, EYE = _host_prep(center, cov_inv_sqrt, weight, threshold)

    if _PROGRAM is None:
        _PROGRAM = _build_program()
    nc = _PROGRAM

    in_maps = []
    for k in range(NCORES):
        in_maps.append(
            {
                "X": X[k * NC_ROWS : (k + 1) * NC_ROWS],
                "U": U,
                "SEL0": SEL0,
                "SELR": SELR,
                "EYER": EYE,
                "EYE": EYE,
            }
        )
    res = run_bass_kernel_spmd(nc, in_maps, list(range(NCORES)))
    out = np.concatenate([res.results[k]["out"] for k in range(NCORES)])
    return out.astype(np.float32)
